# revision 24
# baseline (speedup 1.0000x reference)
"""Trainium2 Bass kernel for the DIP module (tone curve + white balance +
contrast-about-mean + 3x3 sharpen blend), data-parallel over batch on 8 cores.

Pipeline per (image, channel), tiles of [128 rows, 512 cols]:
  t' = a*wb * x^g            ACT: Ln -> PSUM, Exp(scale=g, bias=ln(a*wb)) -> SBUF
                             with accum_out giving per-row sums (for the mean)
  cb = mean(t')*(1-a)/a      tiny matmuls + reduce (exact, fp32)
  u  = clip01(t' + cb)       DVE ts (add,max0) -> GPSIMD ts (min1) -> fp16
  G  = 8s*u - s*(8-neighborhood sum)   fp16 matmuls/tile into PSUM
                             (tridiagonal matrices + halo-row matrices; halo rows
                              pre-summed horizontally on GPSIMD)
  y  = clip01(u + G)         DVE tensor_tensor add + ts clip01, DMA out
"""

import numpy as np

try:
    import concourse.bass as bass
except ImportError:  # pragma: no cover
    import sys

    sys.path.insert(0, "/opt/trn_rl_repo")
    import concourse.bass as bass

from contextlib import ExitStack

import concourse.bacc as bacc
import concourse.tile as tile
from concourse import mybir
from concourse.bass_utils import run_bass_kernel_spmd

F32 = mybir.dt.float32
F16 = mybir.dt.float16

B, C, H, W = 32, 3, 512, 512
NCORES = 8
IPC = B // NCORES  # images per core
NT = H // 128  # row tiles per channel
NPIX = H * W


class _Bacc(bacc.Bacc):
    """Bacc that pins Exp/Ln to the combined table set so the kernel does a
    single ACT_TABLE_LOAD instead of thrashing between exp/ln sets."""

    def insert_act_table_loads(self):
        import bass_rust as _bass_rust

        from concourse.hw_specs import get_activation_tables

        has_activation = any(
            isinstance(i, mybir.InstActivation)
            for b in self.main_func.blocks
            for i in b.instructions
        )
        if not has_activation:
            return
        AF = mybir.ActivationFunctionType
        tables = []
        for name, funcs in get_activation_tables(self.m.arch).items():
            if name != "natural_log_exp_and_others":
                funcs = funcs - {AF.Exp, AF.Ln}
            tables.append((name, funcs))
        _bass_rust.insert_act_table_loads(self, tables)


def _build_program():
    nc = _Bacc("TRN2", target_bir_lowering=False)

    x_in = nc.declare_dram_parameter("x_in", [IPC, C, H, W], F32, isOutput=False)
    # [K row, image, {side, mid}, M row]
    mats = nc.declare_dram_parameter("mats", [128, IPC, 2, 128], F16, isOutput=False)
    # [halo idx, image, tile, M row]
    hmats = nc.declare_dram_parameter("hmats", [6, IPC, NT, 128], F16, isOutput=False)
    emat = nc.declare_dram_parameter("emat", [128, IPC, 128], F32, isOutput=False)
    gcol = nc.declare_dram_parameter("gcol", [128, IPC], F32, isOutput=False)
    scal = nc.declare_dram_parameter("scal", [128, IPC * C], F32, isOutput=False)
    y_out = nc.declare_dram_parameter("y_out", [IPC, C, H, W], F32, isOutput=True)

    AF = mybir.ActivationFunctionType
    ALU = mybir.AluOpType
    AX = mybir.AxisListType

    with ExitStack() as ctx:
        tc = ctx.enter_context(tile.TileContext(nc))
        singles = ctx.enter_context(tc.tile_pool(name="singles", bufs=1))
        xp = ctx.enter_context(tc.tile_pool(name="xp", bufs=3))
        lnp = ctx.enter_context(tc.tile_pool(name="lnp", bufs=3))
        tpp = ctx.enter_context(tc.tile_pool(name="tpp", bufs=3))
        w1p = ctx.enter_context(tc.tile_pool(name="w1p", bufs=2))
        up = ctx.enter_context(tc.tile_pool(name="up", bufs=3))
        accp = ctx.enter_context(tc.tile_pool(name="accp", bufs=3))
        cbpp = ctx.enter_context(tc.tile_pool(name="cbpp", bufs=1, space="PSUM"))
        cbp = ctx.enter_context(tc.tile_pool(name="cbp", bufs=3))
        halop = ctx.enter_context(tc.tile_pool(name="halop", bufs=3))
        hsump = ctx.enter_context(tc.tile_pool(name="hsump", bufs=3))
        outpp = ctx.enter_context(tc.tile_pool(name="outpp", bufs=7, space="PSUM"))
        dp = ctx.enter_context(tc.tile_pool(name="dp", bufs=2))
        outsp = ctx.enter_context(tc.tile_pool(name="outsp", bufs=3))

        # ---- constants into SBUF (one DMA each) ----
        mats_sb = singles.tile([128, IPC, 2, 128], F16)
        nc.sync.dma_start(out=mats_sb[:, :, :, :], in_=mats[:, :, :, :])
        hmats_sb = singles.tile([6, IPC, NT, 128], F16)
        nc.sync.dma_start(out=hmats_sb[:, :, :, :], in_=hmats[:, :, :, :])
        emat_sb = singles.tile([128, IPC, 128], F32)
        nc.sync.dma_start(out=emat_sb[:, :, :], in_=emat[:, :, :])
        gcol_sb = singles.tile([128, IPC], F32)
        nc.sync.dma_start(out=gcol_sb[:, :], in_=gcol[:, :])
        scal_sb = singles.tile([128, IPC * C], F32)
        nc.sync.dma_start(out=scal_sb[:, :], in_=scal[:, :])

        def phase_a(i, c):
            ch = i * C + c
            st = {}
            st["i"], st["c"] = i, c
            acc = accp.tile([128, 1], F32)
            # One big x load per channel, issued from ACT so the WAR
            # against Ln is same-engine (keeps DMA waits low).
            xb = xp.tile([128, NT, 512], F32)
            xsrc = x_in[i, c, :, :].rearrange("(k p) j -> p k j", p=128)
            nc.scalar.dma_start(out=xb[:, :, :], in_=xsrc)
            tpc = tpp.tile([128, NT, 512], F32)
            lt = lnp.tile([128, NT, 512], F32)
            nc.scalar.activation(out=lt[:, :, :], in_=xb[:, :, :], func=AF.Ln)
            nc.scalar.activation(
                out=tpc[:, :, :],
                in_=lt[:, :, :],
                func=AF.Exp,
                scale=gcol_sb[:, i : i + 1],
                bias=scal_sb[:, ch : ch + 1],
                accum_out=acc[:, 0:1],
            )
            # ---- mean -> cb column: cb[m] = const * sum_p acc[p] ----
            cbps = cbpp.tile([128, 1], F32)
            nc.tensor.matmul(
                out=cbps[:, :],
                lhsT=emat_sb[:, i, :],
                rhs=acc[:, :],
                start=True,
                stop=True,
            )
            cb = cbp.tile([128, 1], F32)
            nc.vector.tensor_copy(out=cb[:, :], in_=cbps[:, :])
            st["tpc"], st["cb"] = tpc, cb
            return st

        def phase_b(st):
            i, c = st["i"], st["c"]
            tpc, cb = st["tpc"], st["cb"]
            # ---- u = clip01(t' + cb) in fp16, whole channel per op ----
            w1c = w1p.tile([128, NT, 512], F32)
            nc.vector.tensor_scalar(
                w1c[:, :, :], tpc[:, :, :], cb[:, 0:1], 0.0, ALU.add, ALU.max
            )
            uc = up.tile([128, NT, 512], F16)
            nc.vector.tensor_scalar(uc[:, :, :], w1c[:, :, :], 1.0, None, ALU.min)

            # ---- halo rows {127,128, 255,256, 383,384}: gather + 3-sum ----
            halo = halop.tile([6, 512], F16)
            pairs = [(0, 0, 127), (1, 1, 0), (2, 1, 127), (3, 2, 0), (4, 2, 127), (5, 3, 0)]
            for idx, k, p in pairs:
                nc.gpsimd.dma_start(
                    out=halo[idx : idx + 1, :], in_=uc[p : p + 1, k, :]
                )
            hpair = hsump.tile([6, 512], F16, tag="hpair")
            hs = hsump.tile([6, 512], F16, tag="hs")
            nc.vector.tensor_add(hpair[:, 0:511], halo[:, 0:511], halo[:, 1:512])
            nc.vector.tensor_add(hs[:, 1:511], hpair[:, 0:510], halo[:, 2:512])
            nc.vector.tensor_copy(out=hs[:, 0:512:511], in_=hpair[:, 0:511:510])

            # ---- conv matmuls + blend + clip + store ----
            dc = dp.tile([128, NT, 512], F32)
            oc = outsp.tile([128, NT, 512], F32)
            mmid = mats_sb[:, i, 1, :]
            mside = mats_sb[:, i, 0, :]
            for k in range(NT):
                ob = outpp.tile([128, 512], F32)
                nc.tensor.matmul(
                    out=ob[:, 0:512],
                    lhsT=mmid,
                    rhs=uc[:, k, 0:512],
                    start=True,
                    stop=False,
                )
                nc.tensor.matmul(
                    out=ob[:, 1:512],
                    lhsT=mside,
                    rhs=uc[:, k, 0:511],
                    start=False,
                    stop=False,
                )
                nc.tensor.matmul(
                    out=ob[:, 0:511],
                    lhsT=mside,
                    rhs=uc[:, k, 1:512],
                    start=False,
                    stop=False,
                )
                nc.tensor.matmul(
                    out=ob[:, 0:512],
                    lhsT=hmats_sb[0:6, i, k, :],
                    rhs=hs[:, 0:512],
                    start=False,
                    stop=True,
                )
                nc.vector.tensor_add(dc[:, k, :], uc[:, k, :], ob[:, :])
            nc.vector.tensor_scalar(
                oc[:, :, :], dc[:, :, :], 0.0, 1.0, ALU.max, ALU.min
            )
            ydst = y_out[i, c, :, :].rearrange("(k p) j -> p k j", p=128)
            nc.sync.dma_start(out=ydst, in_=oc[:, :, :])

        chans = [(i, c) for i in range(IPC) for c in range(C)]
        prev = None
        for i, c in chans:
            st = phase_a(i, c)
            if prev is not None:
                phase_b(prev)
            prev = st
        phase_b(prev)
    nc.compile()
    return nc


def _host_inputs(x, gamma, wb, contrast, sharpen_strength):
    """Build per-core input maps (numpy only)."""
    in_maps = []
    onesv = np.ones((128, 1), np.float32)
    for cid in range(NCORES):
        b0 = cid * IPC
        mats = np.zeros((128, IPC, 2, 128), np.float16)
        hmats = np.zeros((6, IPC, NT, 128), np.float16)
        emat = np.zeros((128, IPC, 128), np.float32)
        gcol = np.zeros((128, IPC), np.float32)
        scal = np.zeros((128, IPC * C), np.float32)
        for i in range(IPC):
            b = b0 + i
            a = float(contrast[b])
            s = float(sharpen_strength[b])
            g = float(gamma[b])
            ns = np.float16(-s)
            c8 = np.float16(8.0 * s)
            # mats[:, i, 0] = Mside (all -s taps), mats[:, i, 1] = Mmid (center 8s)
            for m in range(128):
                for dp_ in (-1, 0, 1):
                    p = m + dp_
                    if 0 <= p < 128:
                        mats[p, i, 0, m] = ns
                        mats[p, i, 1, m] = c8 if dp_ == 0 else ns
            # halo rows {127,128,255,256,383,384}: tile k's top neighbor row
            # 128k-1 is halo idx 2(k-1); bottom neighbor 128k+128 is 2k+1
            for k in range(NT):
                if k >= 1:
                    hmats[2 * (k - 1), i, k, 0] = ns
                if k <= 2:
                    hmats[2 * k + 1, i, k, 127] = ns
            emat[:, i, :] = (1.0 - a) / (a * NPIX)
            gcol[:, i] = g
            for c in range(C):
                scal[:, i * C + c] = np.log(a * float(wb[b, c]))
        in_maps.append(
            {
                "x_in": np.ascontiguousarray(x[b0 : b0 + IPC]).astype(
                    np.float32, copy=False
                ),
                "mats": mats,
                "hmats": hmats,
                "emat": emat,
                "gcol": gcol,
                "scal": scal,
            }
        )
    return in_maps


_PROGRAM_CACHE = {}


def kernel(x, gamma, wb, contrast, sharpen_strength):
    x = np.asarray(x, dtype=np.float32)
    gamma = np.asarray(gamma, dtype=np.float32)
    wb = np.asarray(wb, dtype=np.float32)
    contrast = np.asarray(contrast, dtype=np.float32)
    sharpen_strength = np.asarray(sharpen_strength, dtype=np.float32)

    if "nc" not in _PROGRAM_CACHE:
        _PROGRAM_CACHE["nc"] = _build_program()
    nc = _PROGRAM_CACHE["nc"]

    in_maps = _host_inputs(x, gamma, wb, contrast, sharpen_strength)
    res = run_bass_kernel_spmd(nc, in_maps, list(range(NCORES)))
    out = np.empty((B, C, H, W), np.float32)
    for cid in range(NCORES):
        out[cid * IPC : (cid + 1) * IPC] = res.results[cid]["y_out"]
    return out


# revision 25
# speedup vs baseline: 1.0821x; 1.0821x over previous
"""Trainium2 Bass kernel for the DIP module (tone curve + white balance +
contrast-about-mean + 3x3 sharpen blend), data-parallel over batch on 8 cores.

Pipeline per (image, channel), tiles of [128 rows, 512 cols]:
  t' = a*wb * x^g            ACT: Ln -> PSUM, Exp(scale=g, bias=ln(a*wb)) -> SBUF
                             with accum_out giving per-row sums (for the mean)
  cb = mean(t')*(1-a)/a      tiny matmuls + reduce (exact, fp32)
  u  = clip01(t' + cb)       DVE ts (add,max0) -> GPSIMD ts (min1) -> fp16
  G  = 8s*u - s*(8-neighborhood sum)   fp16 matmuls/tile into PSUM
                             (tridiagonal matrices + halo-row matrices; halo rows
                              pre-summed horizontally on GPSIMD)
  y  = clip01(u + G)         DVE tensor_tensor add + ts clip01, DMA out
"""

import numpy as np

try:
    import concourse.bass as bass
except ImportError:  # pragma: no cover
    import sys

    sys.path.insert(0, "/opt/trn_rl_repo")
    import concourse.bass as bass

from contextlib import ExitStack

import concourse.bacc as bacc
import concourse.tile as tile
from concourse import mybir
from concourse.bass_utils import run_bass_kernel_spmd

F32 = mybir.dt.float32
F16 = mybir.dt.float16

B, C, H, W = 32, 3, 512, 512
PRECISE = True  # True: exact +u16 add via DVE TT; False: fold (1+8s) into matmul
NCORES = 8
IPC = B // NCORES  # images per core
NT = H // 128  # row tiles per channel
NPIX = H * W


class _Bacc(bacc.Bacc):
    """Bacc that pins Exp/Ln to the combined table set so the kernel does a
    single ACT_TABLE_LOAD instead of thrashing between exp/ln sets."""

    def insert_act_table_loads(self):
        import bass_rust as _bass_rust

        from concourse.hw_specs import get_activation_tables

        has_activation = any(
            isinstance(i, mybir.InstActivation)
            for b in self.main_func.blocks
            for i in b.instructions
        )
        if not has_activation:
            return
        AF = mybir.ActivationFunctionType
        tables = []
        for name, funcs in get_activation_tables(self.m.arch).items():
            if name != "natural_log_exp_and_others":
                funcs = funcs - {AF.Exp, AF.Ln}
            tables.append((name, funcs))
        _bass_rust.insert_act_table_loads(self, tables)


def _build_program():
    nc = _Bacc("TRN2", target_bir_lowering=False)

    x_in = nc.declare_dram_parameter("x_in", [IPC, C, H, W], F32, isOutput=False)
    # [K row, image, {side, mid}, M row]
    mats = nc.declare_dram_parameter("mats", [128, IPC, 2, 128], F16, isOutput=False)
    # [halo idx, image, tile, M row]
    hmats = nc.declare_dram_parameter("hmats", [6, IPC, NT, 128], F16, isOutput=False)
    emat = nc.declare_dram_parameter("emat", [128, IPC, 128], F32, isOutput=False)
    gcol = nc.declare_dram_parameter("gcol", [128, IPC], F32, isOutput=False)
    scal = nc.declare_dram_parameter("scal", [128, IPC * C], F32, isOutput=False)
    y_out = nc.declare_dram_parameter("y_out", [IPC, C, H, W], F32, isOutput=True)

    AF = mybir.ActivationFunctionType
    ALU = mybir.AluOpType
    AX = mybir.AxisListType

    with ExitStack() as ctx:
        tc = ctx.enter_context(tile.TileContext(nc))
        singles = ctx.enter_context(tc.tile_pool(name="singles", bufs=1))
        xp = ctx.enter_context(tc.tile_pool(name="xp", bufs=3))
        lnp = ctx.enter_context(tc.tile_pool(name="lnp", bufs=3))
        tpp = ctx.enter_context(tc.tile_pool(name="tpp", bufs=3))
        w1p = ctx.enter_context(tc.tile_pool(name="w1p", bufs=2))
        up = ctx.enter_context(tc.tile_pool(name="up", bufs=3))
        accp = ctx.enter_context(tc.tile_pool(name="accp", bufs=3))
        cbpp = ctx.enter_context(tc.tile_pool(name="cbpp", bufs=1, space="PSUM"))
        cbp = ctx.enter_context(tc.tile_pool(name="cbp", bufs=3))
        halop = ctx.enter_context(tc.tile_pool(name="halop", bufs=3))
        hsump = ctx.enter_context(tc.tile_pool(name="hsump", bufs=3))
        outpp = ctx.enter_context(tc.tile_pool(name="outpp", bufs=7, space="PSUM"))
        dp = ctx.enter_context(tc.tile_pool(name="dp", bufs=2))
        outsp = ctx.enter_context(tc.tile_pool(name="outsp", bufs=3))

        # ---- constants into SBUF (one DMA each) ----
        mats_sb = singles.tile([128, IPC, 2, 128], F16)
        nc.sync.dma_start(out=mats_sb[:, :, :, :], in_=mats[:, :, :, :])
        hmats_sb = singles.tile([6, IPC, NT, 128], F16)
        nc.sync.dma_start(out=hmats_sb[:, :, :, :], in_=hmats[:, :, :, :])
        emat_sb = singles.tile([128, IPC, 128], F32)
        nc.sync.dma_start(out=emat_sb[:, :, :], in_=emat[:, :, :])
        gcol_sb = singles.tile([128, IPC], F32)
        nc.sync.dma_start(out=gcol_sb[:, :], in_=gcol[:, :])
        scal_sb = singles.tile([128, IPC * C], F32)
        nc.sync.dma_start(out=scal_sb[:, :], in_=scal[:, :])

        def phase_a(i, c):
            ch = i * C + c
            st = {}
            st["i"], st["c"] = i, c
            acc = accp.tile([128, 1], F32)
            # One big x load per channel, issued from ACT so the WAR
            # against Ln is same-engine (keeps DMA waits low).
            xb = xp.tile([128, NT, 512], F32)
            xsrc = x_in[i, c, :, :].rearrange("(k p) j -> p k j", p=128)
            nc.scalar.dma_start(out=xb[:, :, :], in_=xsrc)
            tpc = tpp.tile([128, NT, 512], F32)
            lt = lnp.tile([128, NT, 512], F32)
            nc.scalar.activation(out=lt[:, :, :], in_=xb[:, :, :], func=AF.Ln)
            nc.scalar.activation(
                out=tpc[:, :, :],
                in_=lt[:, :, :],
                func=AF.Exp,
                scale=gcol_sb[:, i : i + 1],
                bias=scal_sb[:, ch : ch + 1],
                accum_out=acc[:, 0:1],
            )
            # ---- mean -> cb column: cb[m] = const * sum_p acc[p] ----
            cbps = cbpp.tile([128, 1], F32)
            nc.tensor.matmul(
                out=cbps[:, :],
                lhsT=emat_sb[:, i, :],
                rhs=acc[:, :],
                start=True,
                stop=True,
            )
            cb = cbp.tile([128, 1], F32)
            nc.vector.tensor_copy(out=cb[:, :], in_=cbps[:, :])
            st["tpc"], st["cb"] = tpc, cb
            return st

        def phase_b(st):
            i, c = st["i"], st["c"]
            tpc, cb = st["tpc"], st["cb"]
            # ---- u = clip01(t' + cb) in fp16, whole channel per op ----
            w1c = w1p.tile([128, NT, 512], F32)
            nc.vector.tensor_scalar(
                w1c[:, :, :], tpc[:, :, :], cb[:, 0:1], 0.0, ALU.add, ALU.max
            )
            uc = up.tile([128, NT, 512], F16)
            nc.vector.tensor_scalar(uc[:, :, :], w1c[:, :, :], 1.0, None, ALU.min)

            # ---- halo rows {127,128, 255,256, 383,384}: gather + 3-sum ----
            halo = halop.tile([6, 512], F16)
            pairs = [(0, 0, 127), (1, 1, 0), (2, 1, 127), (3, 2, 0), (4, 2, 127), (5, 3, 0)]
            for idx, k, p in pairs:
                nc.sync.dma_start(out=halo[idx : idx + 1, :], in_=uc[p : p + 1, k, :])
            hpair = hsump.tile([6, 512], F16, tag="hpair")
            hs = hsump.tile([6, 512], F16, tag="hs")
            nc.vector.tensor_add(hpair[:, 0:511], halo[:, 0:511], halo[:, 1:512])
            nc.vector.tensor_add(hs[:, 1:511], hpair[:, 0:510], halo[:, 2:512])
            nc.vector.tensor_copy(out=hs[:, 0:512:511], in_=hpair[:, 0:511:510])

            # ---- conv matmuls + blend + clip + store ----
            dc = dp.tile([128, NT, 512], F32)
            oc = outsp.tile([128, NT, 512], F32)
            mmid = mats_sb[:, i, 1, :]
            mside = mats_sb[:, i, 0, :]
            for k in range(NT):
                ob = outpp.tile([128, 512], F32)
                nc.tensor.matmul(
                    out=ob[:, 0:512],
                    lhsT=mmid,
                    rhs=uc[:, k, 0:512],
                    start=True,
                    stop=False,
                )
                nc.tensor.matmul(
                    out=ob[:, 1:512],
                    lhsT=mside,
                    rhs=uc[:, k, 0:511],
                    start=False,
                    stop=False,
                )
                nc.tensor.matmul(
                    out=ob[:, 0:511],
                    lhsT=mside,
                    rhs=uc[:, k, 1:512],
                    start=False,
                    stop=False,
                )
                nc.tensor.matmul(
                    out=ob[:, 0:512],
                    lhsT=hmats_sb[0:6, i, k, :],
                    rhs=hs[:, 0:512],
                    start=False,
                    stop=True,
                )
                if PRECISE:
                    nc.vector.tensor_add(dc[:, k, :], uc[:, k, :], ob[:, :])
                else:
                    nc.vector.tensor_scalar(
                        oc[:, k, :], ob[:, :], 0.0, 1.0, ALU.max, ALU.min
                    )
            if PRECISE:
                nc.vector.tensor_scalar(
                    oc[:, :, :], dc[:, :, :], 0.0, 1.0, ALU.max, ALU.min
                )
            ydst = y_out[i, c, :, :].rearrange("(k p) j -> p k j", p=128)
            nc.sync.dma_start(out=ydst, in_=oc[:, :, :])

        chans = [(i, c) for i in range(IPC) for c in range(C)]
        prev = None
        for i, c in chans:
            st = phase_a(i, c)
            if prev is not None:
                phase_b(prev)
            prev = st
        phase_b(prev)
    nc.compile()
    return nc


def _host_inputs(x, gamma, wb, contrast, sharpen_strength):
    """Build per-core input maps (numpy only)."""
    in_maps = []
    onesv = np.ones((128, 1), np.float32)
    for cid in range(NCORES):
        b0 = cid * IPC
        mats = np.zeros((128, IPC, 2, 128), np.float16)
        hmats = np.zeros((6, IPC, NT, 128), np.float16)
        emat = np.zeros((128, IPC, 128), np.float32)
        gcol = np.zeros((128, IPC), np.float32)
        scal = np.zeros((128, IPC * C), np.float32)
        for i in range(IPC):
            b = b0 + i
            a = float(contrast[b])
            s = float(sharpen_strength[b])
            g = float(gamma[b])
            ns = np.float16(-s)
            c8 = np.float16(8.0 * s) if PRECISE else np.float16(1.0 + 8.0 * s)
            # mats[:, i, 0] = Mside (all -s taps), mats[:, i, 1] = Mmid (center 8s)
            for m in range(128):
                for dp_ in (-1, 0, 1):
                    p = m + dp_
                    if 0 <= p < 128:
                        mats[p, i, 0, m] = ns
                        mats[p, i, 1, m] = c8 if dp_ == 0 else ns
            # halo rows {127,128,255,256,383,384}: tile k's top neighbor row
            # 128k-1 is halo idx 2(k-1); bottom neighbor 128k+128 is 2k+1
            for k in range(NT):
                if k >= 1:
                    hmats[2 * (k - 1), i, k, 0] = ns
                if k <= 2:
                    hmats[2 * k + 1, i, k, 127] = ns
            emat[:, i, :] = (1.0 - a) / (a * NPIX)
            gcol[:, i] = g
            for c in range(C):
                scal[:, i * C + c] = np.log(a * float(wb[b, c]))
        in_maps.append(
            {
                "x_in": np.ascontiguousarray(x[b0 : b0 + IPC]).astype(
                    np.float32, copy=False
                ),
                "mats": mats,
                "hmats": hmats,
                "emat": emat,
                "gcol": gcol,
                "scal": scal,
            }
        )
    return in_maps


_PROGRAM_CACHE = {}


def kernel(x, gamma, wb, contrast, sharpen_strength):
    x = np.asarray(x, dtype=np.float32)
    gamma = np.asarray(gamma, dtype=np.float32)
    wb = np.asarray(wb, dtype=np.float32)
    contrast = np.asarray(contrast, dtype=np.float32)
    sharpen_strength = np.asarray(sharpen_strength, dtype=np.float32)

    if "nc" not in _PROGRAM_CACHE:
        _PROGRAM_CACHE["nc"] = _build_program()
    nc = _PROGRAM_CACHE["nc"]

    in_maps = _host_inputs(x, gamma, wb, contrast, sharpen_strength)
    res = run_bass_kernel_spmd(nc, in_maps, list(range(NCORES)))
    out = np.empty((B, C, H, W), np.float32)
    for cid in range(NCORES):
        out[cid * IPC : (cid + 1) * IPC] = res.results[cid]["y_out"]
    return out


# revision 26
# speedup vs baseline: 1.1692x; 1.0805x over previous
"""Trainium2 Bass kernel for the DIP module (tone curve + white balance +
contrast-about-mean + 3x3 sharpen blend), data-parallel over batch on 8 cores.

Pipeline per (image, channel), tiles of [128 rows, 512 cols]:
  t' = a*wb * x^g            ACT: Ln -> PSUM, Exp(scale=g, bias=ln(a*wb)) -> SBUF
                             with accum_out giving per-row sums (for the mean)
  cb = mean(t')*(1-a)/a      tiny matmuls + reduce (exact, fp32)
  u  = clip01(t' + cb)       DVE ts (add,max0) -> GPSIMD ts (min1) -> fp16
  G  = 8s*u - s*(8-neighborhood sum)   fp16 matmuls/tile into PSUM
                             (tridiagonal matrices + halo-row matrices; halo rows
                              pre-summed horizontally on GPSIMD)
  y  = clip01(u + G)         DVE tensor_tensor add + ts clip01, DMA out
"""

import numpy as np

try:
    import concourse.bass as bass
except ImportError:  # pragma: no cover
    import sys

    sys.path.insert(0, "/opt/trn_rl_repo")
    import concourse.bass as bass

from contextlib import ExitStack

import concourse.bacc as bacc
import concourse.tile as tile
from concourse import mybir
from concourse.bass_utils import run_bass_kernel_spmd

F32 = mybir.dt.float32
F16 = mybir.dt.float16

B, C, H, W = 32, 3, 512, 512
PRECISE = False  # True: exact +u16 add via DVE TT; False: fold (1+8s) into matmul
NCORES = 8
IPC = B // NCORES  # images per core
NT = H // 128  # row tiles per channel
NPIX = H * W


class _Bacc(bacc.Bacc):
    """Bacc that pins Exp/Ln to the combined table set so the kernel does a
    single ACT_TABLE_LOAD instead of thrashing between exp/ln sets."""

    def insert_act_table_loads(self):
        import bass_rust as _bass_rust

        from concourse.hw_specs import get_activation_tables

        has_activation = any(
            isinstance(i, mybir.InstActivation)
            for b in self.main_func.blocks
            for i in b.instructions
        )
        if not has_activation:
            return
        AF = mybir.ActivationFunctionType
        tables = []
        for name, funcs in get_activation_tables(self.m.arch).items():
            if name != "natural_log_exp_and_others":
                funcs = funcs - {AF.Exp, AF.Ln}
            tables.append((name, funcs))
        _bass_rust.insert_act_table_loads(self, tables)


def _build_program():
    nc = _Bacc("TRN2", target_bir_lowering=False)

    x_in = nc.declare_dram_parameter("x_in", [IPC, C, H, W], F32, isOutput=False)
    # [K row, image, {side, mid}, M row]
    mats = nc.declare_dram_parameter("mats", [128, IPC, 2, 128], F16, isOutput=False)
    # [halo idx, image, tile, M row]
    hmats = nc.declare_dram_parameter("hmats", [6, IPC, NT, 128], F16, isOutput=False)
    emat = nc.declare_dram_parameter("emat", [128, IPC, 128], F32, isOutput=False)
    gcol = nc.declare_dram_parameter("gcol", [128, IPC], F32, isOutput=False)
    scal = nc.declare_dram_parameter("scal", [128, IPC * C], F32, isOutput=False)
    y_out = nc.declare_dram_parameter("y_out", [IPC, C, H, W], F32, isOutput=True)

    AF = mybir.ActivationFunctionType
    ALU = mybir.AluOpType
    AX = mybir.AxisListType

    with ExitStack() as ctx:
        tc = ctx.enter_context(tile.TileContext(nc))
        singles = ctx.enter_context(tc.tile_pool(name="singles", bufs=1))
        xp = ctx.enter_context(tc.tile_pool(name="xp", bufs=3))
        lnp = ctx.enter_context(tc.tile_pool(name="lnp", bufs=3))
        tpp = ctx.enter_context(tc.tile_pool(name="tpp", bufs=3))
        w1p = ctx.enter_context(tc.tile_pool(name="w1p", bufs=2))
        up = ctx.enter_context(tc.tile_pool(name="up", bufs=3))
        accp = ctx.enter_context(tc.tile_pool(name="accp", bufs=3))
        cbpp = ctx.enter_context(tc.tile_pool(name="cbpp", bufs=1, space="PSUM"))
        cbp = ctx.enter_context(tc.tile_pool(name="cbp", bufs=3))
        halop = ctx.enter_context(tc.tile_pool(name="halop", bufs=3))
        hsump = ctx.enter_context(tc.tile_pool(name="hsump", bufs=3))
        outpp = ctx.enter_context(tc.tile_pool(name="outpp", bufs=7, space="PSUM"))
        dp = ctx.enter_context(tc.tile_pool(name="dp", bufs=2))
        outsp = ctx.enter_context(tc.tile_pool(name="outsp", bufs=3))

        # ---- constants into SBUF (one DMA each) ----
        mats_sb = singles.tile([128, IPC, 2, 128], F16)
        nc.sync.dma_start(out=mats_sb[:, :, :, :], in_=mats[:, :, :, :])
        hmats_sb = singles.tile([6, IPC, NT, 128], F16)
        nc.sync.dma_start(out=hmats_sb[:, :, :, :], in_=hmats[:, :, :, :])
        emat_sb = singles.tile([128, IPC, 128], F32)
        nc.sync.dma_start(out=emat_sb[:, :, :], in_=emat[:, :, :])
        gcol_sb = singles.tile([128, IPC], F32)
        nc.sync.dma_start(out=gcol_sb[:, :], in_=gcol[:, :])
        scal_sb = singles.tile([128, IPC * C], F32)
        nc.sync.dma_start(out=scal_sb[:, :], in_=scal[:, :])

        def phase_a(i, c):
            ch = i * C + c
            st = {}
            st["i"], st["c"] = i, c
            acc = accp.tile([128, 1], F32)
            # One big x load per channel, issued from ACT so the WAR
            # against Ln is same-engine (keeps DMA waits low).
            xb = xp.tile([128, NT, 512], F32)
            xsrc = x_in[i, c, :, :].rearrange("(k p) j -> p k j", p=128)
            nc.scalar.dma_start(out=xb[:, :, :], in_=xsrc)
            tpc = tpp.tile([128, NT, 512], F32)
            lt = lnp.tile([128, NT, 512], F32)
            nc.scalar.activation(out=lt[:, :, :], in_=xb[:, :, :], func=AF.Ln)
            nc.scalar.activation(
                out=tpc[:, :, :],
                in_=lt[:, :, :],
                func=AF.Exp,
                scale=gcol_sb[:, i : i + 1],
                bias=scal_sb[:, ch : ch + 1],
                accum_out=acc[:, 0:1],
            )
            # ---- mean -> cb column: cb[m] = const * sum_p acc[p] ----
            cbps = cbpp.tile([128, 1], F32)
            nc.tensor.matmul(
                out=cbps[:, :],
                lhsT=emat_sb[:, i, :],
                rhs=acc[:, :],
                start=True,
                stop=True,
            )
            cb = cbp.tile([128, 1], F32)
            nc.vector.tensor_copy(out=cb[:, :], in_=cbps[:, :])
            st["tpc"], st["cb"] = tpc, cb
            return st

        def phase_b(st):
            i, c = st["i"], st["c"]
            tpc, cb = st["tpc"], st["cb"]
            # ---- u = clip01(t' + cb) in fp16, whole channel per op ----
            w1c = w1p.tile([128, NT, 512], F32)
            nc.vector.tensor_scalar(
                w1c[:, :, :], tpc[:, :, :], cb[:, 0:1], 0.0, ALU.add, ALU.max
            )
            uc = up.tile([128, NT, 512], F16)
            nc.vector.tensor_scalar(uc[:, :, :], w1c[:, :, :], 1.0, None, ALU.min)

            # ---- halo rows {127,128, 255,256, 383,384}: gather + 3-sum ----
            halo = halop.tile([6, 512], F16)
            pairs = [(0, 0, 127), (1, 1, 0), (2, 1, 127), (3, 2, 0), (4, 2, 127), (5, 3, 0)]
            for idx, k, p in pairs:
                nc.sync.dma_start(out=halo[idx : idx + 1, :], in_=uc[p : p + 1, k, :])
            hpair = hsump.tile([6, 512], F16, tag="hpair")
            hs = hsump.tile([6, 512], F16, tag="hs")
            nc.vector.tensor_add(hpair[:, 0:511], halo[:, 0:511], halo[:, 1:512])
            nc.vector.tensor_add(hs[:, 1:511], hpair[:, 0:510], halo[:, 2:512])
            nc.vector.tensor_copy(out=hs[:, 0:512:511], in_=hpair[:, 0:511:510])

            # ---- conv matmuls + blend + clip + store ----
            dc = dp.tile([128, NT, 512], F32)
            oc = outsp.tile([128, NT, 512], F32)
            mmid = mats_sb[:, i, 1, :]
            mside = mats_sb[:, i, 0, :]
            for k in range(NT):
                ob = outpp.tile([128, 512], F32)
                nc.tensor.matmul(
                    out=ob[:, 0:512],
                    lhsT=mmid,
                    rhs=uc[:, k, 0:512],
                    start=True,
                    stop=False,
                )
                nc.tensor.matmul(
                    out=ob[:, 1:512],
                    lhsT=mside,
                    rhs=uc[:, k, 0:511],
                    start=False,
                    stop=False,
                )
                nc.tensor.matmul(
                    out=ob[:, 0:511],
                    lhsT=mside,
                    rhs=uc[:, k, 1:512],
                    start=False,
                    stop=False,
                )
                nc.tensor.matmul(
                    out=ob[:, 0:512],
                    lhsT=hmats_sb[0:6, i, k, :],
                    rhs=hs[:, 0:512],
                    start=False,
                    stop=True,
                )
                if PRECISE:
                    nc.vector.tensor_add(dc[:, k, :], uc[:, k, :], ob[:, :])
                else:
                    nc.vector.tensor_scalar(
                        oc[:, k, :], ob[:, :], 0.0, 1.0, ALU.max, ALU.min
                    )
            if PRECISE:
                nc.vector.tensor_scalar(
                    oc[:, :, :], dc[:, :, :], 0.0, 1.0, ALU.max, ALU.min
                )
            ydst = y_out[i, c, :, :].rearrange("(k p) j -> p k j", p=128)
            nc.sync.dma_start(out=ydst, in_=oc[:, :, :])

        chans = [(i, c) for i in range(IPC) for c in range(C)]
        prev = None
        for i, c in chans:
            st = phase_a(i, c)
            if prev is not None:
                phase_b(prev)
            prev = st
        phase_b(prev)
    nc.compile()
    return nc


def _host_inputs(x, gamma, wb, contrast, sharpen_strength):
    """Build per-core input maps (numpy only)."""
    in_maps = []
    onesv = np.ones((128, 1), np.float32)
    for cid in range(NCORES):
        b0 = cid * IPC
        mats = np.zeros((128, IPC, 2, 128), np.float16)
        hmats = np.zeros((6, IPC, NT, 128), np.float16)
        emat = np.zeros((128, IPC, 128), np.float32)
        gcol = np.zeros((128, IPC), np.float32)
        scal = np.zeros((128, IPC * C), np.float32)
        for i in range(IPC):
            b = b0 + i
            a = float(contrast[b])
            s = float(sharpen_strength[b])
            g = float(gamma[b])
            ns = np.float16(-s)
            c8 = np.float16(8.0 * s) if PRECISE else np.float16(1.0 + 8.0 * s)
            # mats[:, i, 0] = Mside (all -s taps), mats[:, i, 1] = Mmid (center 8s)
            for m in range(128):
                for dp_ in (-1, 0, 1):
                    p = m + dp_
                    if 0 <= p < 128:
                        mats[p, i, 0, m] = ns
                        mats[p, i, 1, m] = c8 if dp_ == 0 else ns
            # halo rows {127,128,255,256,383,384}: tile k's top neighbor row
            # 128k-1 is halo idx 2(k-1); bottom neighbor 128k+128 is 2k+1
            for k in range(NT):
                if k >= 1:
                    hmats[2 * (k - 1), i, k, 0] = ns
                if k <= 2:
                    hmats[2 * k + 1, i, k, 127] = ns
            emat[:, i, :] = (1.0 - a) / (a * NPIX)
            gcol[:, i] = g
            for c in range(C):
                scal[:, i * C + c] = np.log(a * float(wb[b, c]))
        in_maps.append(
            {
                "x_in": np.ascontiguousarray(x[b0 : b0 + IPC]).astype(
                    np.float32, copy=False
                ),
                "mats": mats,
                "hmats": hmats,
                "emat": emat,
                "gcol": gcol,
                "scal": scal,
            }
        )
    return in_maps


_PROGRAM_CACHE = {}


def kernel(x, gamma, wb, contrast, sharpen_strength):
    x = np.asarray(x, dtype=np.float32)
    gamma = np.asarray(gamma, dtype=np.float32)
    wb = np.asarray(wb, dtype=np.float32)
    contrast = np.asarray(contrast, dtype=np.float32)
    sharpen_strength = np.asarray(sharpen_strength, dtype=np.float32)

    if "nc" not in _PROGRAM_CACHE:
        _PROGRAM_CACHE["nc"] = _build_program()
    nc = _PROGRAM_CACHE["nc"]

    in_maps = _host_inputs(x, gamma, wb, contrast, sharpen_strength)
    res = run_bass_kernel_spmd(nc, in_maps, list(range(NCORES)))
    out = np.empty((B, C, H, W), np.float32)
    for cid in range(NCORES):
        out[cid * IPC : (cid + 1) * IPC] = res.results[cid]["y_out"]
    return out


# revision 28
# speedup vs baseline: 1.1713x; 1.0019x over previous
"""Trainium2 Bass kernel for the DIP module (tone curve + white balance +
contrast-about-mean + 3x3 sharpen blend), data-parallel over batch on 8 cores.

Pipeline per (image, channel), tiles of [128 rows, 512 cols]:
  t' = a*wb * x^g            ACT: Ln -> PSUM, Exp(scale=g, bias=ln(a*wb)) -> SBUF
                             with accum_out giving per-row sums (for the mean)
  cb = mean(t')*(1-a)/a      tiny matmuls + reduce (exact, fp32)
  u  = clip01(t' + cb)       DVE ts (add,max0) -> GPSIMD ts (min1) -> fp16
  G  = 8s*u - s*(8-neighborhood sum)   fp16 matmuls/tile into PSUM
                             (tridiagonal matrices + halo-row matrices; halo rows
                              pre-summed horizontally on GPSIMD)
  y  = clip01(u + G)         DVE tensor_tensor add + ts clip01, DMA out
"""

import numpy as np

try:
    import concourse.bass as bass
except ImportError:  # pragma: no cover
    import sys

    sys.path.insert(0, "/opt/trn_rl_repo")
    import concourse.bass as bass

from contextlib import ExitStack

import concourse.bacc as bacc
import concourse.tile as tile
from concourse import mybir
from concourse.bass_utils import run_bass_kernel_spmd

F32 = mybir.dt.float32
F16 = mybir.dt.float16

B, C, H, W = 32, 3, 512, 512
PRECISE = False  # True: exact +u16 add via DVE TT; False: fold (1+8s) into matmul
NCORES = 8
IPC = B // NCORES  # images per core
NT = H // 128  # row tiles per channel
NPIX = H * W


class _Bacc(bacc.Bacc):
    """Bacc that pins Exp/Ln to the combined table set so the kernel does a
    single ACT_TABLE_LOAD instead of thrashing between exp/ln sets."""

    def insert_act_table_loads(self):
        import bass_rust as _bass_rust

        from concourse.hw_specs import get_activation_tables

        has_activation = any(
            isinstance(i, mybir.InstActivation)
            for b in self.main_func.blocks
            for i in b.instructions
        )
        if not has_activation:
            return
        AF = mybir.ActivationFunctionType
        tables = []
        for name, funcs in get_activation_tables(self.m.arch).items():
            if name != "natural_log_exp_and_others":
                funcs = funcs - {AF.Exp, AF.Ln}
            tables.append((name, funcs))
        _bass_rust.insert_act_table_loads(self, tables)


def _build_program(slotmask):
    nc = _Bacc("TRN2", target_bir_lowering=False)

    x_in = nc.declare_dram_parameter("x_in", [IPC, C, H, W], F32, isOutput=False)
    # [K row, image, {side, mid}, M row]
    mats = nc.declare_dram_parameter("mats", [128, IPC, 2, 128], F16, isOutput=False)
    # [halo idx, image, tile, M row]
    hmats = nc.declare_dram_parameter("hmats", [6, IPC, NT, 128], F16, isOutput=False)
    emat = nc.declare_dram_parameter("emat", [128, IPC, 128], F32, isOutput=False)
    gcol = nc.declare_dram_parameter("gcol", [128, IPC], F32, isOutput=False)
    scal = nc.declare_dram_parameter("scal", [128, IPC * C], F32, isOutput=False)
    y_out = nc.declare_dram_parameter("y_out", [IPC, C, H, W], F32, isOutput=True)

    AF = mybir.ActivationFunctionType
    ALU = mybir.AluOpType
    AX = mybir.AxisListType

    with ExitStack() as ctx:
        tc = ctx.enter_context(tile.TileContext(nc))
        singles = ctx.enter_context(tc.tile_pool(name="singles", bufs=1))
        xp = ctx.enter_context(tc.tile_pool(name="xp", bufs=3))
        lnp = ctx.enter_context(tc.tile_pool(name="lnp", bufs=3))
        tpp = ctx.enter_context(tc.tile_pool(name="tpp", bufs=4))
        w1p = ctx.enter_context(tc.tile_pool(name="w1p", bufs=2))
        up = ctx.enter_context(tc.tile_pool(name="up", bufs=4))
        accp = ctx.enter_context(tc.tile_pool(name="accp", bufs=4))
        cbpp = ctx.enter_context(tc.tile_pool(name="cbpp", bufs=1, space="PSUM"))
        cbp = ctx.enter_context(tc.tile_pool(name="cbp", bufs=4))
        halop = ctx.enter_context(tc.tile_pool(name="halop", bufs=3))
        hsump = ctx.enter_context(tc.tile_pool(name="hsump", bufs=3))
        outpp = ctx.enter_context(tc.tile_pool(name="outpp", bufs=7, space="PSUM"))
        dp = ctx.enter_context(tc.tile_pool(name="dp", bufs=2))
        outsp = ctx.enter_context(tc.tile_pool(name="outsp", bufs=3))

        # ---- constants into SBUF (one DMA each) ----
        mats_sb = singles.tile([128, IPC, 2, 128], F16)
        nc.sync.dma_start(out=mats_sb[:, :, :, :], in_=mats[:, :, :, :])
        hmats_sb = singles.tile([6, IPC, NT, 128], F16)
        nc.sync.dma_start(out=hmats_sb[:, :, :, :], in_=hmats[:, :, :, :])
        emat_sb = singles.tile([128, IPC, 128], F32)
        nc.sync.dma_start(out=emat_sb[:, :, :], in_=emat[:, :, :])
        gcol_sb = singles.tile([128, IPC], F32)
        nc.sync.dma_start(out=gcol_sb[:, :], in_=gcol[:, :])
        scal_sb = singles.tile([128, IPC * C], F32)
        nc.sync.dma_start(out=scal_sb[:, :], in_=scal[:, :])

        def phase_a(i, c):
            ch = i * C + c
            st = {}
            st["i"], st["c"] = i, c
            acc = accp.tile([128, 1], F32)
            # One big x load per channel, issued from ACT so the WAR
            # against Ln is same-engine (keeps DMA waits low).
            xb = xp.tile([128, NT, 512], F32)
            xsrc = x_in[i, c, :, :].rearrange("(k p) j -> p k j", p=128)
            nc.scalar.dma_start(out=xb[:, :, :], in_=xsrc)
            tpc = tpp.tile([128, NT, 512], F32)
            lt = lnp.tile([128, NT, 512], F32)
            nc.scalar.activation(out=lt[:, :, :], in_=xb[:, :, :], func=AF.Ln)
            nc.scalar.activation(
                out=tpc[:, :, :],
                in_=lt[:, :, :],
                func=AF.Exp,
                scale=gcol_sb[:, i : i + 1],
                bias=scal_sb[:, ch : ch + 1],
                accum_out=acc[:, 0:1],
            )
            # ---- mean -> cb column: cb[m] = const * sum_p acc[p] ----
            cbps = cbpp.tile([128, 1], F32)
            nc.tensor.matmul(
                out=cbps[:, :],
                lhsT=emat_sb[:, i, :],
                rhs=acc[:, :],
                start=True,
                stop=True,
            )
            cb = cbp.tile([128, 1], F32)
            nc.vector.tensor_copy(out=cb[:, :], in_=cbps[:, :])
            st["tpc"], st["cb"] = tpc, cb
            return st

        def phase_b(st):
            i, c = st["i"], st["c"]
            tpc, cb = st["tpc"], st["cb"]
            # ---- u = clip01(t' + cb) in fp16, whole channel per op ----
            uc = up.tile([128, NT, 512], F16)
            if slotmask[i]:
                # a > 1 -> cb < 0: need the max(.,0)
                w1c = w1p.tile([128, NT, 512], F32)
                nc.vector.tensor_scalar(
                    w1c[:, :, :], tpc[:, :, :], cb[:, 0:1], 0.0, ALU.add, ALU.max
                )
                nc.vector.tensor_scalar(
                    uc[:, :, :], w1c[:, :, :], 1.0, None, ALU.min
                )
            else:
                # a <= 1 -> cb >= 0 and t' >= 0: max(.,0) is a no-op
                nc.vector.tensor_scalar(
                    uc[:, :, :], tpc[:, :, :], cb[:, 0:1], 1.0, ALU.add, ALU.min
                )

            # ---- halo rows: [0:3]=rows{127,255,383}, [3:6]=rows{128,256,384} ----
            halo = halop.tile([6, 512], F16)
            nc.sync.dma_start(out=halo[0:3, :], in_=uc[127:128, 0:3, :])
            nc.sync.dma_start(out=halo[3:6, :], in_=uc[0:1, 1:4, :])
            hpair = hsump.tile([6, 512], F16, tag="hpair")
            hs = hsump.tile([6, 512], F16, tag="hs")
            nc.vector.tensor_add(hpair[:, 0:511], halo[:, 0:511], halo[:, 1:512])
            nc.vector.tensor_add(hs[:, 1:511], hpair[:, 0:510], halo[:, 2:512])
            nc.vector.tensor_copy(out=hs[:, 0:512:511], in_=hpair[:, 0:511:510])

            # ---- conv matmuls + blend + clip + store ----
            dc = dp.tile([128, NT, 512], F32)
            oc = outsp.tile([128, NT, 512], F32)
            mmid = mats_sb[:, i, 1, :]
            mside = mats_sb[:, i, 0, :]
            for k in range(NT):
                ob = outpp.tile([128, 512], F32)
                nc.tensor.matmul(
                    out=ob[:, 0:512],
                    lhsT=mmid,
                    rhs=uc[:, k, 0:512],
                    start=True,
                    stop=False,
                )
                nc.tensor.matmul(
                    out=ob[:, 1:512],
                    lhsT=mside,
                    rhs=uc[:, k, 0:511],
                    start=False,
                    stop=False,
                )
                nc.tensor.matmul(
                    out=ob[:, 0:511],
                    lhsT=mside,
                    rhs=uc[:, k, 1:512],
                    start=False,
                    stop=False,
                )
                nc.tensor.matmul(
                    out=ob[:, 0:512],
                    lhsT=hmats_sb[0:6, i, k, :],
                    rhs=hs[:, 0:512],
                    start=False,
                    stop=True,
                )
                if PRECISE:
                    nc.vector.tensor_add(dc[:, k, :], uc[:, k, :], ob[:, :])
                else:
                    nc.vector.tensor_scalar(
                        oc[:, k, :], ob[:, :], 0.0, 1.0, ALU.max, ALU.min
                    )
            if PRECISE:
                nc.vector.tensor_scalar(
                    oc[:, :, :], dc[:, :, :], 0.0, 1.0, ALU.max, ALU.min
                )
            ydst = y_out[i, c, :, :].rearrange("(k p) j -> p k j", p=128)
            nc.sync.dma_start(out=ydst, in_=oc[:, :, :])

        chans = [(i, c) for i in range(IPC) for c in range(C)]
        prev = None
        for i, c in chans:
            st = phase_a(i, c)
            if prev is not None:
                phase_b(prev)
            prev = st
        phase_b(prev)
    nc.compile()
    return nc


def _host_inputs(x, gamma, wb, contrast, sharpen_strength, idx):
    """Build per-core input maps (numpy only). idx[cid][i] = global image."""
    in_maps = []
    for cid in range(NCORES):
        imgs = idx[cid]
        mats = np.zeros((128, IPC, 2, 128), np.float16)
        hmats = np.zeros((6, IPC, NT, 128), np.float16)
        emat = np.zeros((128, IPC, 128), np.float32)
        gcol = np.zeros((128, IPC), np.float32)
        scal = np.zeros((128, IPC * C), np.float32)
        for i in range(IPC):
            b = imgs[i]
            a = float(contrast[b])
            s = float(sharpen_strength[b])
            g = float(gamma[b])
            ns = np.float16(-s)
            c8 = np.float16(8.0 * s) if PRECISE else np.float16(1.0 + 8.0 * s)
            # mats[:, i, 0] = Mside (all -s taps), mats[:, i, 1] = Mmid (center 8s)
            for m in range(128):
                for dp_ in (-1, 0, 1):
                    p = m + dp_
                    if 0 <= p < 128:
                        mats[p, i, 0, m] = ns
                        mats[p, i, 1, m] = c8 if dp_ == 0 else ns
            # halo rows {127,128,255,256,383,384}: tile k's top neighbor row
            # 128k-1 is halo idx 2(k-1); bottom neighbor 128k+128 is 2k+1
            for k in range(NT):
                if k >= 1:
                    hmats[k - 1, i, k, 0] = ns
                if k <= 2:
                    hmats[3 + k, i, k, 127] = ns
            emat[:, i, :] = (1.0 - a) / (a * NPIX)
            gcol[:, i] = g
            for c in range(C):
                scal[:, i * C + c] = np.log(a * float(wb[b, c]))
        in_maps.append(
            {
                "x_in": np.ascontiguousarray(x[imgs]).astype(np.float32, copy=False),
                "mats": mats,
                "hmats": hmats,
                "emat": emat,
                "gcol": gcol,
                "scal": scal,
            }
        )
    return in_maps


_PROGRAM_CACHE = {}


def kernel(x, gamma, wb, contrast, sharpen_strength):
    x = np.asarray(x, dtype=np.float32)
    gamma = np.asarray(gamma, dtype=np.float32)
    wb = np.asarray(wb, dtype=np.float32)
    contrast = np.asarray(contrast, dtype=np.float32)
    sharpen_strength = np.asarray(sharpen_strength, dtype=np.float32)

    # Sort images by contrast and stripe across cores so slot i is
    # homogeneous in sign(1-a); the single-op clip path is only legal
    # when every image in the slot has a <= 1 (SPMD: shared program).
    order = np.argsort(contrast, kind="stable")
    idx = [[int(order[i * NCORES + cid]) for i in range(IPC)] for cid in range(NCORES)]
    slotmask = tuple(
        bool(any(contrast[order[i * NCORES + cid]] > 1.0 for cid in range(NCORES)))
        for i in range(IPC)
    )
    if slotmask not in _PROGRAM_CACHE:
        _PROGRAM_CACHE.clear()
        _PROGRAM_CACHE[slotmask] = _build_program(slotmask)
    nc = _PROGRAM_CACHE[slotmask]

    in_maps = _host_inputs(x, gamma, wb, contrast, sharpen_strength, idx)
    res = run_bass_kernel_spmd(nc, in_maps, list(range(NCORES)))
    out = np.empty((B, C, H, W), np.float32)
    for cid in range(NCORES):
        for i in range(IPC):
            out[idx[cid][i]] = res.results[cid]["y_out"][i]
    return out


# revision 29
# speedup vs baseline: 1.2528x; 1.0695x over previous
"""Trainium2 Bass kernel for the DIP module (tone curve + white balance +
contrast-about-mean + 3x3 sharpen blend), data-parallel over batch on 8 cores.

Pipeline per (image, channel), tiles of [128 rows, 512 cols]:
  t' = a*wb * x^g            ACT: Ln -> PSUM, Exp(scale=g, bias=ln(a*wb)) -> SBUF
                             with accum_out giving per-row sums (for the mean)
  cb = mean(t')*(1-a)/a      tiny matmuls + reduce (exact, fp32)
  u  = clip01(t' + cb)       DVE ts (add,max0) -> GPSIMD ts (min1) -> fp16
  G  = 8s*u - s*(8-neighborhood sum)   fp16 matmuls/tile into PSUM
                             (tridiagonal matrices + halo-row matrices; halo rows
                              pre-summed horizontally on GPSIMD)
  y  = clip01(u + G)         DVE tensor_tensor add + ts clip01, DMA out
"""

import numpy as np

try:
    import concourse.bass as bass
except ImportError:  # pragma: no cover
    import sys

    sys.path.insert(0, "/opt/trn_rl_repo")
    import concourse.bass as bass

from contextlib import ExitStack

import concourse.bacc as bacc
import concourse.tile as tile
from concourse import mybir
from concourse.bass_utils import run_bass_kernel_spmd

F32 = mybir.dt.float32
F16 = mybir.dt.float16

B, C, H, W = 32, 3, 512, 512
PRECISE = False  # True: exact +u16 add via DVE TT; False: fold (1+8s) into matmul
NCORES = 8
IPC = B // NCORES  # images per core
NT = H // 128  # row tiles per channel
NPIX = H * W


class _Bacc(bacc.Bacc):
    """Bacc that pins Exp/Ln to the combined table set so the kernel does a
    single ACT_TABLE_LOAD instead of thrashing between exp/ln sets."""

    def insert_act_table_loads(self):
        import bass_rust as _bass_rust

        from concourse.hw_specs import get_activation_tables

        has_activation = any(
            isinstance(i, mybir.InstActivation)
            for b in self.main_func.blocks
            for i in b.instructions
        )
        if not has_activation:
            return
        AF = mybir.ActivationFunctionType
        tables = []
        for name, funcs in get_activation_tables(self.m.arch).items():
            if name != "natural_log_exp_and_others":
                funcs = funcs - {AF.Exp, AF.Ln}
            tables.append((name, funcs))
        _bass_rust.insert_act_table_loads(self, tables)


_dma_engines = {}


def _build_program(slotmask):
    nc = _Bacc("TRN2", target_bir_lowering=False)
    _dma_engines[nc] = [nc.scalar, nc.sync, nc.gpsimd]

    x_in = nc.declare_dram_parameter("x_in", [IPC, C, H, W], F32, isOutput=False)
    # [K row, image, {side, mid}, M row]
    mats = nc.declare_dram_parameter("mats", [128, IPC, 2, 128], F16, isOutput=False)
    # [halo idx, image, tile, M row]
    hmats = nc.declare_dram_parameter("hmats", [6, IPC, NT, 128], F16, isOutput=False)
    emat = nc.declare_dram_parameter("emat", [128, IPC, 128], F32, isOutput=False)
    gcol = nc.declare_dram_parameter("gcol", [128, IPC], F32, isOutput=False)
    scal = nc.declare_dram_parameter("scal", [128, IPC * C], F32, isOutput=False)
    y_out = nc.declare_dram_parameter("y_out", [IPC, C, H, W], F32, isOutput=True)

    AF = mybir.ActivationFunctionType
    ALU = mybir.AluOpType
    AX = mybir.AxisListType

    with ExitStack() as ctx:
        tc = ctx.enter_context(tile.TileContext(nc))
        singles = ctx.enter_context(tc.tile_pool(name="singles", bufs=1))
        xp = ctx.enter_context(tc.tile_pool(name="xp", bufs=3))
        lnp = ctx.enter_context(tc.tile_pool(name="lnp", bufs=3))
        tpp = ctx.enter_context(tc.tile_pool(name="tpp", bufs=4))
        w1p = ctx.enter_context(tc.tile_pool(name="w1p", bufs=2))
        up = ctx.enter_context(tc.tile_pool(name="up", bufs=4))
        accp = ctx.enter_context(tc.tile_pool(name="accp", bufs=4))
        cbpp = ctx.enter_context(tc.tile_pool(name="cbpp", bufs=1, space="PSUM"))
        cbp = ctx.enter_context(tc.tile_pool(name="cbp", bufs=4))
        halop = ctx.enter_context(tc.tile_pool(name="halop", bufs=3))
        hsump = ctx.enter_context(tc.tile_pool(name="hsump", bufs=3))
        outpp = ctx.enter_context(tc.tile_pool(name="outpp", bufs=7, space="PSUM"))
        dp = ctx.enter_context(tc.tile_pool(name="dp", bufs=2))
        outsp = ctx.enter_context(tc.tile_pool(name="outsp", bufs=3))

        # ---- constants into SBUF (one DMA each) ----
        mats_sb = singles.tile([128, IPC, 2, 128], F16)
        nc.sync.dma_start(out=mats_sb[:, :, :, :], in_=mats[:, :, :, :])
        hmats_sb = singles.tile([6, IPC, NT, 128], F16)
        nc.sync.dma_start(out=hmats_sb[:, :, :, :], in_=hmats[:, :, :, :])
        emat_sb = singles.tile([128, IPC, 128], F32)
        nc.sync.dma_start(out=emat_sb[:, :, :], in_=emat[:, :, :])
        gcol_sb = singles.tile([128, IPC], F32)
        nc.sync.dma_start(out=gcol_sb[:, :], in_=gcol[:, :])
        scal_sb = singles.tile([128, IPC * C], F32)
        nc.sync.dma_start(out=scal_sb[:, :], in_=scal[:, :])

        def phase_a(i, c, qrr):
            ch = i * C + c
            st = {}
            st["i"], st["c"], st["qrr"] = i, c, qrr
            acc = accp.tile([128, 1], F32)
            # x load split into halves, spread across DMA dispatch queues
            # (each HWDGE/SWDGE queue dispatches ~128GB/s; spreading raises
            # aggregate DMA bandwidth).
            xb = xp.tile([128, NT, 512], F32)
            xsrc = x_in[i, c, :, :].rearrange("(k p) j -> p k j", p=128)
            e1 = _dma_engines[nc][st["qrr"] % 3]
            e2 = _dma_engines[nc][(st["qrr"] + 1) % 3]
            e1.dma_start(out=xb[:, 0:2, :], in_=xsrc[:, 0:2, :])
            e2.dma_start(out=xb[:, 2:4, :], in_=xsrc[:, 2:4, :])
            tpc = tpp.tile([128, NT, 512], F32)
            lt = lnp.tile([128, NT, 512], F32)
            nc.scalar.activation(out=lt[:, :, :], in_=xb[:, :, :], func=AF.Ln)
            nc.scalar.activation(
                out=tpc[:, :, :],
                in_=lt[:, :, :],
                func=AF.Exp,
                scale=gcol_sb[:, i : i + 1],
                bias=scal_sb[:, ch : ch + 1],
                accum_out=acc[:, 0:1],
            )
            # ---- mean -> cb column: cb[m] = const * sum_p acc[p] ----
            cbps = cbpp.tile([128, 1], F32)
            nc.tensor.matmul(
                out=cbps[:, :],
                lhsT=emat_sb[:, i, :],
                rhs=acc[:, :],
                start=True,
                stop=True,
            )
            cb = cbp.tile([128, 1], F32)
            nc.vector.tensor_copy(out=cb[:, :], in_=cbps[:, :])
            st["tpc"], st["cb"] = tpc, cb
            return st

        def phase_b(st):
            i, c = st["i"], st["c"]
            tpc, cb = st["tpc"], st["cb"]
            # ---- u = clip01(t' + cb) in fp16, whole channel per op ----
            uc = up.tile([128, NT, 512], F16)
            if slotmask[i]:
                # a > 1 -> cb < 0: need the max(.,0)
                w1c = w1p.tile([128, NT, 512], F32)
                nc.vector.tensor_scalar(
                    w1c[:, :, :], tpc[:, :, :], cb[:, 0:1], 0.0, ALU.add, ALU.max
                )
                nc.vector.tensor_scalar(
                    uc[:, :, :], w1c[:, :, :], 1.0, None, ALU.min
                )
            else:
                # a <= 1 -> cb >= 0 and t' >= 0: max(.,0) is a no-op
                nc.vector.tensor_scalar(
                    uc[:, :, :], tpc[:, :, :], cb[:, 0:1], 1.0, ALU.add, ALU.min
                )

            # ---- halo rows: [0:3]=rows{127,255,383}, [3:6]=rows{128,256,384} ----
            halo = halop.tile([6, 512], F16)
            nc.sync.dma_start(out=halo[0:3, :], in_=uc[127:128, 0:3, :])
            nc.sync.dma_start(out=halo[3:6, :], in_=uc[0:1, 1:4, :])
            hpair = hsump.tile([6, 512], F16, tag="hpair")
            hs = hsump.tile([6, 512], F16, tag="hs")
            nc.vector.tensor_add(hpair[:, 0:511], halo[:, 0:511], halo[:, 1:512])
            nc.vector.tensor_add(hs[:, 1:511], hpair[:, 0:510], halo[:, 2:512])
            nc.vector.tensor_copy(out=hs[:, 0:512:511], in_=hpair[:, 0:511:510])

            # ---- conv matmuls + blend + clip + store ----
            dc = dp.tile([128, NT, 512], F32)
            oc = outsp.tile([128, NT, 512], F32)
            mmid = mats_sb[:, i, 1, :]
            mside = mats_sb[:, i, 0, :]
            for k in range(NT):
                ob = outpp.tile([128, 512], F32)
                nc.tensor.matmul(
                    out=ob[:, 0:512],
                    lhsT=mmid,
                    rhs=uc[:, k, 0:512],
                    start=True,
                    stop=False,
                )
                nc.tensor.matmul(
                    out=ob[:, 1:512],
                    lhsT=mside,
                    rhs=uc[:, k, 0:511],
                    start=False,
                    stop=False,
                )
                nc.tensor.matmul(
                    out=ob[:, 0:511],
                    lhsT=mside,
                    rhs=uc[:, k, 1:512],
                    start=False,
                    stop=False,
                )
                nc.tensor.matmul(
                    out=ob[:, 0:512],
                    lhsT=hmats_sb[0:6, i, k, :],
                    rhs=hs[:, 0:512],
                    start=False,
                    stop=True,
                )
                if PRECISE:
                    nc.vector.tensor_add(dc[:, k, :], uc[:, k, :], ob[:, :])
                else:
                    nc.vector.tensor_scalar(
                        oc[:, k, :], ob[:, :], 0.0, 1.0, ALU.max, ALU.min
                    )
            if PRECISE:
                nc.vector.tensor_scalar(
                    oc[:, :, :], dc[:, :, :], 0.0, 1.0, ALU.max, ALU.min
                )
            ydst = y_out[i, c, :, :].rearrange("(k p) j -> p k j", p=128)
            e3 = _dma_engines[nc][(st["qrr"] + 2) % 3]
            e4 = _dma_engines[nc][st["qrr"] % 3]
            e3.dma_start(out=ydst[:, 0:2, :], in_=oc[:, 0:2, :])
            e4.dma_start(out=ydst[:, 2:4, :], in_=oc[:, 2:4, :])

        chans = [(i, c) for i in range(IPC) for c in range(C)]
        prev = None
        for n_, (i, c) in enumerate(chans):
            st = phase_a(i, c, n_)
            if prev is not None:
                phase_b(prev)
            prev = st
        phase_b(prev)
    nc.compile()
    return nc


def _host_inputs(x, gamma, wb, contrast, sharpen_strength, idx):
    """Build per-core input maps (numpy only). idx[cid][i] = global image."""
    in_maps = []
    for cid in range(NCORES):
        imgs = idx[cid]
        mats = np.zeros((128, IPC, 2, 128), np.float16)
        hmats = np.zeros((6, IPC, NT, 128), np.float16)
        emat = np.zeros((128, IPC, 128), np.float32)
        gcol = np.zeros((128, IPC), np.float32)
        scal = np.zeros((128, IPC * C), np.float32)
        for i in range(IPC):
            b = imgs[i]
            a = float(contrast[b])
            s = float(sharpen_strength[b])
            g = float(gamma[b])
            ns = np.float16(-s)
            c8 = np.float16(8.0 * s) if PRECISE else np.float16(1.0 + 8.0 * s)
            # mats[:, i, 0] = Mside (all -s taps), mats[:, i, 1] = Mmid (center 8s)
            for m in range(128):
                for dp_ in (-1, 0, 1):
                    p = m + dp_
                    if 0 <= p < 128:
                        mats[p, i, 0, m] = ns
                        mats[p, i, 1, m] = c8 if dp_ == 0 else ns
            # halo rows {127,128,255,256,383,384}: tile k's top neighbor row
            # 128k-1 is halo idx 2(k-1); bottom neighbor 128k+128 is 2k+1
            for k in range(NT):
                if k >= 1:
                    hmats[k - 1, i, k, 0] = ns
                if k <= 2:
                    hmats[3 + k, i, k, 127] = ns
            emat[:, i, :] = (1.0 - a) / (a * NPIX)
            gcol[:, i] = g
            for c in range(C):
                scal[:, i * C + c] = np.log(a * float(wb[b, c]))
        in_maps.append(
            {
                "x_in": np.ascontiguousarray(x[imgs]).astype(np.float32, copy=False),
                "mats": mats,
                "hmats": hmats,
                "emat": emat,
                "gcol": gcol,
                "scal": scal,
            }
        )
    return in_maps


_PROGRAM_CACHE = {}


def kernel(x, gamma, wb, contrast, sharpen_strength):
    x = np.asarray(x, dtype=np.float32)
    gamma = np.asarray(gamma, dtype=np.float32)
    wb = np.asarray(wb, dtype=np.float32)
    contrast = np.asarray(contrast, dtype=np.float32)
    sharpen_strength = np.asarray(sharpen_strength, dtype=np.float32)

    # Sort images by contrast and stripe across cores so slot i is
    # homogeneous in sign(1-a); the single-op clip path is only legal
    # when every image in the slot has a <= 1 (SPMD: shared program).
    order = np.argsort(contrast, kind="stable")
    idx = [[int(order[i * NCORES + cid]) for i in range(IPC)] for cid in range(NCORES)]
    slotmask = tuple(
        bool(any(contrast[order[i * NCORES + cid]] > 1.0 for cid in range(NCORES)))
        for i in range(IPC)
    )
    if slotmask not in _PROGRAM_CACHE:
        _PROGRAM_CACHE.clear()
        _PROGRAM_CACHE[slotmask] = _build_program(slotmask)
    nc = _PROGRAM_CACHE[slotmask]

    in_maps = _host_inputs(x, gamma, wb, contrast, sharpen_strength, idx)
    res = run_bass_kernel_spmd(nc, in_maps, list(range(NCORES)))
    out = np.empty((B, C, H, W), np.float32)
    for cid in range(NCORES):
        for i in range(IPC):
            out[idx[cid][i]] = res.results[cid]["y_out"][i]
    return out


# revision 32
# speedup vs baseline: 1.2938x; 1.0327x over previous
"""Trainium2 Bass kernel for the DIP module (tone curve + white balance +
contrast-about-mean + 3x3 sharpen blend), data-parallel over batch on 8 cores.

Pipeline per (image, channel), tiles of [128 rows, 512 cols]:
  t' = a*wb * x^g            ACT: Ln -> PSUM, Exp(scale=g, bias=ln(a*wb)) -> SBUF
                             with accum_out giving per-row sums (for the mean)
  cb = mean(t')*(1-a)/a      tiny matmuls + reduce (exact, fp32)
  u  = clip01(t' + cb)       DVE ts (add,max0) -> GPSIMD ts (min1) -> fp16
  G  = 8s*u - s*(8-neighborhood sum)   fp16 matmuls/tile into PSUM
                             (tridiagonal matrices + halo-row matrices; halo rows
                              pre-summed horizontally on GPSIMD)
  y  = clip01(u + G)         DVE tensor_tensor add + ts clip01, DMA out
"""

import numpy as np

try:
    import concourse.bass as bass
except ImportError:  # pragma: no cover
    import sys

    sys.path.insert(0, "/opt/trn_rl_repo")
    import concourse.bass as bass

from contextlib import ExitStack

import concourse.bacc as bacc
import concourse.tile as tile
from concourse import mybir
from concourse.bass_utils import run_bass_kernel_spmd

F32 = mybir.dt.float32
F16 = mybir.dt.float16

B, C, H, W = 32, 3, 512, 512
PRECISE = False  # True: exact +u16 add via DVE TT; False: fold (1+8s) into matmul
NCORES = 8
IPC = B // NCORES  # images per core
NT = H // 128  # row tiles per channel
NPIX = H * W


class _Bacc(bacc.Bacc):
    """Bacc that pins Exp/Ln to the combined table set so the kernel does a
    single ACT_TABLE_LOAD instead of thrashing between exp/ln sets."""

    def insert_act_table_loads(self):
        import bass_rust as _bass_rust

        from concourse.hw_specs import get_activation_tables

        has_activation = any(
            isinstance(i, mybir.InstActivation)
            for b in self.main_func.blocks
            for i in b.instructions
        )
        if not has_activation:
            return
        AF = mybir.ActivationFunctionType
        tables = []
        for name, funcs in get_activation_tables(self.m.arch).items():
            if name != "natural_log_exp_and_others":
                funcs = funcs - {AF.Exp, AF.Ln}
            tables.append((name, funcs))
        _bass_rust.insert_act_table_loads(self, tables)


_dma_engines = {}


def _build_program(slotmask):
    nc = _Bacc("TRN2", target_bir_lowering=False)
    _dma_engines[nc] = [nc.scalar, nc.sync, nc.gpsimd]

    x_in = nc.declare_dram_parameter("x_in", [IPC, C, H, W], F32, isOutput=False)
    # [K row, image, {side, mid}, M row]
    mats = nc.declare_dram_parameter("mats", [128, IPC, 2, 128], F16, isOutput=False)
    # [halo idx, image, tile, M row]
    hmats = nc.declare_dram_parameter("hmats", [6, IPC, NT, 128], F16, isOutput=False)
    emat = nc.declare_dram_parameter("emat", [128, IPC, 128], F32, isOutput=False)
    gcol = nc.declare_dram_parameter("gcol", [128, IPC], F32, isOutput=False)
    scal = nc.declare_dram_parameter("scal", [128, IPC * C], F32, isOutput=False)
    y_out = nc.declare_dram_parameter("y_out", [IPC, C, H, W], F32, isOutput=True)

    AF = mybir.ActivationFunctionType
    ALU = mybir.AluOpType
    AX = mybir.AxisListType

    with ExitStack() as ctx:
        tc = ctx.enter_context(tile.TileContext(nc))
        singles = ctx.enter_context(tc.tile_pool(name="singles", bufs=1))
        xp = ctx.enter_context(tc.tile_pool(name="xp", bufs=3))
        lnp = ctx.enter_context(tc.tile_pool(name="lnp", bufs=2))
        tpp = ctx.enter_context(tc.tile_pool(name="tpp", bufs=5))
        w1p = ctx.enter_context(tc.tile_pool(name="w1p", bufs=2))
        up = ctx.enter_context(tc.tile_pool(name="up", bufs=6))
        accp = ctx.enter_context(tc.tile_pool(name="accp", bufs=4))
        cbpp = ctx.enter_context(tc.tile_pool(name="cbpp", bufs=1, space="PSUM"))
        cbp = ctx.enter_context(tc.tile_pool(name="cbp", bufs=4))
        halop = ctx.enter_context(tc.tile_pool(name="halop", bufs=3))
        hsump = ctx.enter_context(tc.tile_pool(name="hsump", bufs=3))
        outpp = ctx.enter_context(tc.tile_pool(name="outpp", bufs=7, space="PSUM"))
        dp = ctx.enter_context(tc.tile_pool(name="dp", bufs=2 if PRECISE else 1))
        outsp = ctx.enter_context(tc.tile_pool(name="outsp", bufs=3))

        # ---- constants into SBUF (one DMA each) ----
        mats_sb = singles.tile([128, IPC, 2, 128], F16)
        nc.gpsimd.dma_start(out=mats_sb[:, :, :, :], in_=mats[:, :, :, :])
        hmats_sb = singles.tile([6, IPC, NT, 128], F16)
        nc.gpsimd.dma_start(out=hmats_sb[:, :, :, :], in_=hmats[:, :, :, :])
        emat_sb = singles.tile([128, IPC, 128], F32)
        nc.gpsimd.dma_start(out=emat_sb[:, :, :], in_=emat[:, :, :])
        gcol_sb = singles.tile([128, IPC], F32)
        nc.gpsimd.dma_start(out=gcol_sb[:, :], in_=gcol[:, :])
        scal_sb = singles.tile([128, IPC * C], F32)
        nc.gpsimd.dma_start(out=scal_sb[:, :], in_=scal[:, :])

        # ---- PE HAM warm-up: keep TensorE busy through the startup
        # bubble so the first conv matmuls run at 2.4 GHz ----
        wps = cbpp.tile([128, 512], F32, tag="cbps")
        for _ in range(14):
            nc.tensor.matmul(
                out=wps[:, :],
                lhsT=emat_sb[:, 0, :],
                rhs=emat_sb[:, :, :].rearrange("p a b -> p (a b)"),
                start=True,
                stop=True,
            )

        def phase_a(i, c, qrr):
            ch = i * C + c
            st = {}
            st["i"], st["c"], st["qrr"] = i, c, qrr
            acc = accp.tile([128, 1], F32)
            # x load split into halves, spread across DMA dispatch queues
            # (each HWDGE/SWDGE queue dispatches ~128GB/s; spreading raises
            # aggregate DMA bandwidth).
            xb = xp.tile([128, NT, 512], F32)
            xsrc = x_in[i, c, :, :].rearrange("(k p) j -> p k j", p=128)
            e1 = _dma_engines[nc][st["qrr"] % 3]
            e2 = _dma_engines[nc][(st["qrr"] + 1) % 3]
            e1.dma_start(out=xb[:, 0:2, :], in_=xsrc[:, 0:2, :])
            e2.dma_start(out=xb[:, 2:4, :], in_=xsrc[:, 2:4, :])
            tpc = tpp.tile([128, NT, 512], F32)
            lt = lnp.tile([128, NT, 512], F32)
            nc.scalar.activation(out=lt[:, :, :], in_=xb[:, :, :], func=AF.Ln)
            nc.scalar.activation(
                out=tpc[:, :, :],
                in_=lt[:, :, :],
                func=AF.Exp,
                scale=gcol_sb[:, i : i + 1],
                bias=scal_sb[:, ch : ch + 1],
                accum_out=acc[:, 0:1],
            )
            # ---- mean -> cb column: cb[m] = const * sum_p acc[p] ----
            cbps = cbpp.tile([128, 1], F32, tag="cbps")
            nc.tensor.matmul(
                out=cbps[:, :],
                lhsT=emat_sb[:, i, :],
                rhs=acc[:, :],
                start=True,
                stop=True,
            )
            cb = cbp.tile([128, 1], F32)
            nc.vector.tensor_copy(out=cb[:, :], in_=cbps[:, :])
            st["tpc"], st["cb"] = tpc, cb
            return st

        def phase_b(st):
            i, c = st["i"], st["c"]
            tpc, cb = st["tpc"], st["cb"]
            # ---- u = clip01(t' + cb) in fp16, whole channel per op ----
            uc = up.tile([128, NT, 512], F16)
            if slotmask[i]:
                # a > 1 -> cb < 0: need the max(.,0)
                w1c = w1p.tile([128, NT, 512], F32)
                nc.vector.tensor_scalar(
                    w1c[:, :, :], tpc[:, :, :], cb[:, 0:1], 0.0, ALU.add, ALU.max
                )
                nc.vector.tensor_scalar(
                    uc[:, :, :], w1c[:, :, :], 1.0, None, ALU.min
                )
            else:
                # a <= 1 -> cb >= 0 and t' >= 0: max(.,0) is a no-op
                nc.vector.tensor_scalar(
                    uc[:, :, :], tpc[:, :, :], cb[:, 0:1], 1.0, ALU.add, ALU.min
                )

            # ---- halo rows: [0:3]=rows{127,255,383}, [3:6]=rows{128,256,384} ----
            halo = halop.tile([6, 512], F16)
            nc.sync.dma_start(out=halo[0:3, :], in_=uc[127:128, 0:3, :])
            nc.sync.dma_start(out=halo[3:6, :], in_=uc[0:1, 1:4, :])
            hpair = hsump.tile([6, 512], F16, tag="hpair")
            hs = hsump.tile([6, 512], F16, tag="hs")
            nc.vector.tensor_add(hpair[:, 0:511], halo[:, 0:511], halo[:, 1:512])
            nc.vector.tensor_add(hs[:, 1:511], hpair[:, 0:510], halo[:, 2:512])
            nc.vector.tensor_copy(out=hs[:, 0:512:511], in_=hpair[:, 0:511:510])

            # ---- conv matmuls + blend + clip + store ----
            dc = dp.tile([128, NT, 512], F32) if PRECISE else None
            oc = outsp.tile([128, NT, 512], F32)
            mmid = mats_sb[:, i, 1, :]
            mside = mats_sb[:, i, 0, :]
            for k in range(NT):
                ob = outpp.tile([128, 512], F32)
                nc.tensor.matmul(
                    out=ob[:, 0:512],
                    lhsT=mmid,
                    rhs=uc[:, k, 0:512],
                    start=True,
                    stop=False,
                )
                nc.tensor.matmul(
                    out=ob[:, 1:512],
                    lhsT=mside,
                    rhs=uc[:, k, 0:511],
                    start=False,
                    stop=False,
                )
                nc.tensor.matmul(
                    out=ob[:, 0:511],
                    lhsT=mside,
                    rhs=uc[:, k, 1:512],
                    start=False,
                    stop=False,
                )
                nc.tensor.matmul(
                    out=ob[:, 0:512],
                    lhsT=hmats_sb[0:6, i, k, :],
                    rhs=hs[:, 0:512],
                    start=False,
                    stop=True,
                )
                if PRECISE:
                    nc.vector.tensor_add(dc[:, k, :], uc[:, k, :], ob[:, :])
                else:
                    nc.vector.tensor_scalar(
                        oc[:, k, :], ob[:, :], 0.0, 1.0, ALU.max, ALU.min
                    )
            if PRECISE:
                nc.vector.tensor_scalar(
                    oc[:, :, :], dc[:, :, :], 0.0, 1.0, ALU.max, ALU.min
                )
            ydst = y_out[i, c, :, :].rearrange("(k p) j -> p k j", p=128)
            e3 = _dma_engines[nc][(st["qrr"] + 2) % 3]
            e4 = _dma_engines[nc][st["qrr"] % 3]
            e3.dma_start(out=ydst[:, 0:2, :], in_=oc[:, 0:2, :])
            e4.dma_start(out=ydst[:, 2:4, :], in_=oc[:, 2:4, :])

        chans = [(i, c) for i in range(IPC) for c in range(C)]
        prev = None
        for n_, (i, c) in enumerate(chans):
            st = phase_a(i, c, n_)
            if prev is not None:
                phase_b(prev)
            prev = st
        phase_b(prev)
    nc.compile()
    return nc


def _host_inputs(x, gamma, wb, contrast, sharpen_strength, idx):
    """Build per-core input maps (numpy only). idx[cid][i] = global image."""
    in_maps = []
    for cid in range(NCORES):
        imgs = idx[cid]
        mats = np.zeros((128, IPC, 2, 128), np.float16)
        hmats = np.zeros((6, IPC, NT, 128), np.float16)
        emat = np.zeros((128, IPC, 128), np.float32)
        gcol = np.zeros((128, IPC), np.float32)
        scal = np.zeros((128, IPC * C), np.float32)
        for i in range(IPC):
            b = imgs[i]
            a = float(contrast[b])
            s = float(sharpen_strength[b])
            g = float(gamma[b])
            ns = np.float16(-s)
            c8 = np.float16(8.0 * s) if PRECISE else np.float16(1.0 + 8.0 * s)
            # mats[:, i, 0] = Mside (all -s taps), mats[:, i, 1] = Mmid (center 8s)
            for m in range(128):
                for dp_ in (-1, 0, 1):
                    p = m + dp_
                    if 0 <= p < 128:
                        mats[p, i, 0, m] = ns
                        mats[p, i, 1, m] = c8 if dp_ == 0 else ns
            # halo rows {127,128,255,256,383,384}: tile k's top neighbor row
            # 128k-1 is halo idx 2(k-1); bottom neighbor 128k+128 is 2k+1
            for k in range(NT):
                if k >= 1:
                    hmats[k - 1, i, k, 0] = ns
                if k <= 2:
                    hmats[3 + k, i, k, 127] = ns
            emat[:, i, :] = (1.0 - a) / (a * NPIX)
            gcol[:, i] = g
            for c in range(C):
                scal[:, i * C + c] = np.log(a * float(wb[b, c]))
        in_maps.append(
            {
                "x_in": np.ascontiguousarray(x[imgs]).astype(np.float32, copy=False),
                "mats": mats,
                "hmats": hmats,
                "emat": emat,
                "gcol": gcol,
                "scal": scal,
            }
        )
    return in_maps


_PROGRAM_CACHE = {}


def kernel(x, gamma, wb, contrast, sharpen_strength):
    x = np.asarray(x, dtype=np.float32)
    gamma = np.asarray(gamma, dtype=np.float32)
    wb = np.asarray(wb, dtype=np.float32)
    contrast = np.asarray(contrast, dtype=np.float32)
    sharpen_strength = np.asarray(sharpen_strength, dtype=np.float32)

    # Sort images by contrast and stripe across cores so slot i is
    # homogeneous in sign(1-a); the single-op clip path is only legal
    # when every image in the slot has a <= 1 (SPMD: shared program).
    order = np.argsort(contrast, kind="stable")
    idx = [[int(order[i * NCORES + cid]) for i in range(IPC)] for cid in range(NCORES)]
    slotmask = tuple(
        bool(any(contrast[order[i * NCORES + cid]] > 1.0 for cid in range(NCORES)))
        for i in range(IPC)
    )
    if slotmask not in _PROGRAM_CACHE:
        _PROGRAM_CACHE.clear()
        _PROGRAM_CACHE[slotmask] = _build_program(slotmask)
    nc = _PROGRAM_CACHE[slotmask]

    in_maps = _host_inputs(x, gamma, wb, contrast, sharpen_strength, idx)
    res = run_bass_kernel_spmd(nc, in_maps, list(range(NCORES)))
    out = np.empty((B, C, H, W), np.float32)
    for cid in range(NCORES):
        for i in range(IPC):
            out[idx[cid][i]] = res.results[cid]["y_out"][i]
    return out
